# revision 1
# baseline (speedup 1.0000x reference)
"""Causal self-attention Trainium2 kernel.

Problem: B=8, T=1024, C=2048, 16 heads x 128 head-dim, fp32.
Sharding: data-parallel over batch -- each of the 8 NeuronCores computes one
batch element end-to-end; no collectives.

Per-core dataflow (all matmuls contract over the 128-partition dim):
  x [T,C] --PE transpose--> xT [C,T]
  qT = W_q^T @ xT, kT = W_k^T @ xT          (heads' [d,T] layouts, d=128)
  v  = x @ W_v  (natural [T,d]) via lhsT=xT  (spilled to DRAM, re-read per head)
  per head, per 256-wide q-pair:
    sT[k,q]   = kT-block^T-free @ qT-chunk   (scores transposed: k on partitions)
    expsT     = Exp(sT / sqrt(d))  (ACT), causal-masked multiplicatively (DVE)
    yT[d,q]  += v-block^T(lhsT) @ expsT      (PE accumulate)
    den[1,q] += ones^T @ expsT               (PE accumulate)
    yT_norm   = yT * broadcast(1/den)        (K=1 PE outer product + DVE mul)
  out = yT^T @ W_proj + b  (lhsT=yT slices; bias via K=1 matmul with ones)
"""

import math
from contextlib import ExitStack

import numpy as np

import concourse.bass as bass
import concourse.mybir as mybir
import concourse.tile as tile
from concourse.masks import make_identity
from concourse.vector_clock import ScopedClock

F32 = mybir.dt.float32
F32R = mybir.dt.float32r

B, T, C = 8, 1024, 2048
NH, HD = 16, 128
P = 128
TT = T // P            # 8 row tiles
CT = C // P            # 16 channel tiles
QP = 256               # q-pair width (2 row tiles) for fp32r full-rate moving dim
NQP = T // QP          # 4 q-pairs
SM_SCALE = 1.0 / math.sqrt(HD)

N_CORES = 8

# --------------------------------------------------------------------------
# Walrus workaround: this container's walrus rejects any instruction with
# more than one sync wait command. Split multi-wait instructions into a
# chain of single-wait NoOps/Drains on the same engine (engine queues
# process waits in order, so semantics are unchanged).
# --------------------------------------------------------------------------
_orig_commit_instruction = tile.TileContext._commit_instruction


def _patched_commit_instruction(self, inst, lazy_reg_writes=True):
    si = inst.sync_info
    if (
        si is not None
        and len(si.on_wait) > 1
        and inst.engine != mybir.EngineType.Unassigned
    ):
        waits = list(si.on_wait)
        for w in waits[:-1]:
            nop = mybir.InstNoOp(
                name=self.nc.get_next_instruction_name(),
                engine=inst.engine,
                bass_nofuse=True,
                sync_info=mybir.SyncInfo(on_wait=[w], on_update=[]),
            )
            _orig_commit_instruction(self, nop, lazy_reg_writes=False)
        inst.sync_info = mybir.SyncInfo(
            on_wait=[waits[-1]], on_update=list(si.on_update)
        )
    return _orig_commit_instruction(self, inst, lazy_reg_writes=lazy_reg_writes)


def _patched_drain_and_barrier(self, tick_clock, wait_clock):
    drain_inst = self.nc.sync.drain()
    wait_clock.add_sem_waits(
        drain_inst.ins, ScopedClock({None: tick_clock.global_clock})
    )
    si = drain_inst.ins.sync_info
    if si is not None and len(si.on_wait) > 1:
        waits = list(si.on_wait)
        drain_inst.ins.sync_info = mybir.SyncInfo(
            on_wait=[waits[0]], on_update=list(si.on_update)
        )
        for w in waits[1:]:
            d2 = self.nc.sync.drain()
            d2.ins.sync_info = mybir.SyncInfo(on_wait=[w], on_update=[])
    self.nc.all_engine_barrier()
    assert self.sems is not None
    popped = self.nc._tile_sem_poison_stack.pop()
    assert popped is self._sem_poison
    self.nc.clear_and_free_semaphores(list(self.sems.allocated().values()))
    self.nc.all_engine_barrier()


def _apply_patches():
    tile.TileContext._commit_instruction = _patched_commit_instruction
    tile.TileContext._drain_and_barrier = _patched_drain_and_barrier


# --------------------------------------------------------------------------
# Kernel builder
# --------------------------------------------------------------------------

def build_kernel(mode: str = "f32r", repeats: int = 1,
                 rep_phase: str = "all") -> bass.Bass:
    """mode: 'f32r' (fast, TF32-like matmuls) or 'f32' (full fp32).
    repeats: emit the computation N times (timing calibration).
    rep_phase: which phases reps>0 emit: all|ph0|v|attn|proj."""
    _apply_patches()
    mm_dt = F32R if mode == "f32r" else F32

    nc = bass.Bass("TRN2", target_bir_lowering=False, debug=False)

    x_ap = nc.dram_tensor("x", [T, C], F32, kind="ExternalInput").ap()
    wa_ap = nc.dram_tensor("W_attn", [C, 3 * C], F32, kind="ExternalInput").ap()
    ba_ap = nc.dram_tensor("b_attn", [3 * C], F32, kind="ExternalInput").ap()
    wp_ap = nc.dram_tensor("W_proj", [C, C], F32, kind="ExternalInput").ap()
    bp_ap = nc.dram_tensor("b_proj", [C], F32, kind="ExternalInput").ap()
    out_ap = nc.dram_tensor("out", [T, C], F32, kind="ExternalOutput").ap()
    vspill_ap = nc.dram_tensor("v_spill", [T, C], F32).ap()
    yspill_ap = nc.dram_tensor("y_spill", [C, T], F32).ap()

    def r(ap):
        return ap.bitcast(mm_dt) if mm_dt is F32R else ap

    # DRAM views
    x_rows = x_ap.rearrange("(i p) c -> i p c", p=P)          # [TT, P, C]
    out_rows = out_ap.rearrange("(i p) c -> i p c", p=P)      # [TT, P, C]
    wa_3d = wa_ap.rearrange("(j p) n -> p j n", p=P)          # [P, CT, 3C]
    wp_3d = wp_ap.rearrange("(j p) n -> p j n", p=P)          # [P, CT, C]
    vsp_rows = vspill_ap.rearrange("(j p) c -> j p c", p=P)   # [TT, P, C]
    vsp_3d = vspill_ap.rearrange("(j p) c -> p j c", p=P)     # [P, TT, C]
    ysp_rows = yspill_ap.rearrange("(h p) t -> h p t", p=P)   # [NH, P, T]
    ysp_3d = yspill_ap.rearrange("(h p) t -> p h t", p=P)     # [P, NH, T]
    ba_col = ba_ap.rearrange("(n p one) -> n p one", p=P, one=1)  # [48, P, 1]
    bv_row = ba_ap.rearrange("(n c) -> n c", n=3)             # [3, C]
    bp_row = bp_ap.rearrange("(one c) -> one c", one=1)       # [1, C]

    with tile.TileContext(nc) as tc, ExitStack() as ctx:
        # ---------------- constants ----------------
        const = ctx.enter_context(tc.tile_pool(name="const", bufs=1))
        ident = const.tile([P, P], F32)
        make_identity(nc, ident[:])
        # causal masks for the two diagonal k-blocks of each q-pair
        # maskA[k, q] = 1 if q >= k else 0 ; maskB[k, q] = 1 if q >= k+128
        maskA = const.tile([P, QP], F32)
        nc.gpsimd.memset(maskA[:], 1.0)
        nc.gpsimd.affine_select(
            out=maskA[:], in_=maskA[:], compare_op=mybir.AluOpType.is_ge,
            fill=0.0, base=0, pattern=[[1, QP]], channel_multiplier=-1)
        maskB = const.tile([P, QP], F32)
        nc.gpsimd.memset(maskB[:], 1.0)
        nc.gpsimd.affine_select(
            out=maskB[:], in_=maskB[:], compare_op=mybir.AluOpType.is_ge,
            fill=0.0, base=-P, pattern=[[1, QP]], channel_multiplier=-1)
        # ones columns/rows (matmul operands -> mm_dt, produced via DVE copy)
        ones_col_f = const.tile([P, 1], F32)
        nc.vector.memset(ones_col_f[:], 1.0)
        ones_col = const.tile([P, 1], mm_dt)
        nc.vector.tensor_copy(ones_col[:], ones_col_f[:])
        ones_row_f = const.tile([1, P], F32)
        nc.vector.memset(ones_row_f[:], 1.0)
        ones_row = const.tile([1, P], mm_dt)
        nc.vector.tensor_copy(ones_row[:], ones_row_f[:])
        # bias rows for v and proj (K=1 matmul rhs)
        bv_sb = const.tile([1, C], mm_dt)
        nc.sync.dma_start(bv_sb[:], r(bv_row[2:3, :]))
        bp_sb = const.tile([1, C], mm_dt)
        nc.sync.dma_start(bp_sb[:], r(bp_row[:, :]))

        for _rep in range(repeats):
            first = _rep == 0
            do_ph0 = first or rep_phase in ("all", "ph0", "v", "attn")
            do_v = first or rep_phase in ("all", "v")
            do_attn = first or rep_phase in ("all", "attn")
            do_proj = first or rep_phase in ("all", "proj")
            rctx = ctx.enter_context(ExitStack())
            ph12 = rctx.enter_context(ExitStack())
            if do_ph0:
                xT_pool = ph12.enter_context(tc.tile_pool(name="xT", bufs=1))
                xT = [xT_pool.tile([P, T], mm_dt, tag=f"xT{j}", name=f"xT{j}")
                      for j in range(CT)]

            with tc.tile_pool(name="psA", bufs=1, space="PSUM") as psA, \
                 tc.tile_pool(name="ph0", bufs=3) as ph0:
                # ---------------- phase 0: transpose x -> xT ----------------
                for i in range(TT if do_ph0 else 0):
                    xa = ph0.tile([P, C], F32, tag="xa")
                    nc.sync.dma_start(xa[:], x_rows[i])
                    for j in range(CT):
                        tp = psA.tile([P, P], F32, tag="tp", bufs=2)
                        nc.tensor.transpose(
                            tp[:], xa[:, j * P:(j + 1) * P], ident[:])
                        nc.vector.tensor_copy(xT[j][:, i * P:(i + 1) * P], tp[:])

                # ------------- phase 0.5: v = x @ W_v + b_v -> DRAM ----------
                # n-pair inner so each xT lhsT is reused by 2 adjacent matmuls
                for np_i in range(C // 1024 if do_v else 0):
                    wv = []
                    for c in range(CT):
                        wvc = ph0.tile([P, 1024], mm_dt, tag=f"wv{c}", bufs=1,
                                       name=f"wv{c}_{np_i}")
                        nc.sync.dma_start(
                            wvc[:],
                            r(wa_3d[:, c,
                                    2 * C + np_i * 1024:
                                    2 * C + (np_i + 1) * 1024]))
                        wv.append(wvc)
                    for i in range(TT):
                        pv = [psA.tile([P, 512], F32, tag="big", bufs=4,
                                       name=f"pv{np_i}_{i}_{nn}")
                              for nn in range(2)]
                        for c in range(CT):
                            for nn in range(2):
                                nc.tensor.matmul(
                                    pv[nn][:], xT[c][:, i * P:(i + 1) * P],
                                    wv[c][:, nn * 512:(nn + 1) * 512],
                                    start=(c == 0), stop=False)
                        for nn in range(2):
                            n = np_i * 2 + nn
                            nc.tensor.matmul(
                                pv[nn][:], ones_row[:],
                                bv_sb[:, n * 512:(n + 1) * 512],
                                start=False, stop=True)
                            vsb = ph0.tile([P, 512], F32, tag="vout")
                            nc.scalar.activation(
                                vsb[:], pv[nn][:],
                                mybir.ActivationFunctionType.Copy)
                            nc.sync.dma_start(
                                vsp_rows[i][:, n * 512:(n + 1) * 512], vsb[:])

            # ---------------- phase 1+2: per-head attention ----------------
            psB = ph12.enter_context(
                tc.tile_pool(name="psB", bufs=1, space="PSUM"))
            att = ph12.enter_context(tc.tile_pool(name="att", bufs=2))
            exps_pool = ph12.enter_context(tc.tile_pool(name="exps", bufs=22))

            for h in range(NH if do_attn else 0):
                # weights for q,k of this head: [P, CT*P] each
                wq = att.tile([P, C], mm_dt, tag="wq")
                nc.sync.dma_start(
                    wq[:].rearrange("p (j f) -> p j f", f=P),
                    r(wa_3d[:, :, h * P:(h + 1) * P]))
                wk = att.tile([P, C], mm_dt, tag="wk")
                nc.sync.dma_start(
                    wk[:].rearrange("p (j f) -> p j f", f=P),
                    r(wa_3d[:, :, C + h * P: C + (h + 1) * P]))
                bq = att.tile([P, 1], F32, tag="bq")
                nc.sync.dma_start(bq[:], ba_col[h])
                bk = att.tile([P, 1], F32, tag="bk")
                nc.sync.dma_start(bk[:], ba_col[NH + h])

                # qT, kT [P(d), T]; ch-inner so each w lhsT is reused twice
                qT = att.tile([P, T], mm_dt, tag="qT")
                kT = att.tile([P, T], mm_dt, tag="kT")
                for di, (dst, w, bias) in enumerate(
                        ((qT, wq, bq), (kT, wk, bk))):
                    pq = [psB.tile([P, 512], F32, tag="qk", bufs=2,
                                   name=f"pq{h}_{di}_{ch}")
                          for ch in range(T // 512)]
                    for c in range(CT):
                        for ch in range(T // 512):
                            nc.tensor.matmul(
                                pq[ch][:], w[:, c * P:(c + 1) * P],
                                xT[c][:, ch * 512:(ch + 1) * 512],
                                start=(c == 0), stop=(c == CT - 1))
                    for ch in range(T // 512):
                        nc.scalar.activation(
                            dst[:, ch * 512:(ch + 1) * 512], pq[ch][:],
                            mybir.ActivationFunctionType.Identity,
                            bias=bias[:])

                # v for this head: j-th 128-block is v rows [128j:128j+128]
                vh = att.tile([P, T], mm_dt, tag="vh")
                nc.sync.dma_start(
                    vh[:].rearrange("p (j f) -> p j f", f=P),
                    r(vsp_3d[:, :, h * P:(h + 1) * P]))

                yTh = att.tile([P, T], F32, tag="yTh")

                # scores j-outer: one kT lhsT load per k-block
                exps = {}
                for j in range(2 * NQP):
                    for p_i in range(j // 2, NQP):
                        qs = slice(p_i * QP, (p_i + 1) * QP)
                        sT = psB.tile([P, QP], F32, tag="sT", bufs=3,
                                      name=f"sT{h}_{j}_{p_i}")
                        nc.tensor.matmul(
                            sT[:], kT[:, j * P:(j + 1) * P], qT[:, qs],
                            start=True, stop=True)
                        ex = exps_pool.tile([P, QP], mm_dt, tag="exps",
                                            name=f"ex{h}_{j}_{p_i}")
                        nc.scalar.activation(
                            ex[:], sT[:], mybir.ActivationFunctionType.Exp,
                            scale=SM_SCALE)
                        if j == 2 * p_i:
                            nc.vector.tensor_mul(ex[:], ex[:], maskA[:])
                        elif j == 2 * p_i + 1:
                            nc.vector.tensor_mul(ex[:], ex[:], maskB[:])
                        exps[(j, p_i)] = ex

                for p_i in range(NQP):
                    nkt = 2 * p_i + 2
                    qs = slice(p_i * QP, (p_i + 1) * QP)
                    den = psB.tile([1, QP], F32, tag="den", bufs=1,
                                   name=f"den{h}_{p_i}")
                    yacc = psB.tile([P, QP], F32, tag="yacc", bufs=2,
                                    name=f"yacc{h}_{p_i}")
                    for j in range(nkt):
                        nc.tensor.matmul(
                            yacc[:], vh[:, j * P:(j + 1) * P],
                            exps[(j, p_i)][:],
                            start=(j == 0), stop=(j == nkt - 1))
                        nc.tensor.matmul(
                            den[:], ones_col[:], exps[(j, p_i)][:],
                            start=(j == 0), stop=(j == nkt - 1))
                    rden = att.tile([1, QP], mm_dt, tag="rden")
                    with nc.allow_low_precision(
                        reason="fp32r rounding of softmax denom is intentional"
                    ):
                        nc.vector.reciprocal(rden[:], den[:])
                    bc = psB.tile([P, QP], F32, tag="sT", bufs=3,
                                  name=f"bc{h}_{p_i}")
                    nc.tensor.matmul(bc[:], ones_row[:], rden[:],
                                     start=True, stop=True)
                    bc_sb = att.tile([P, QP], F32, tag="bc_sb")
                    nc.scalar.activation(
                        bc_sb[:], bc[:], mybir.ActivationFunctionType.Copy)
                    nc.vector.tensor_mul(yTh[:, qs], yacc[:], bc_sb[:])
                nc.sync.dma_start(ysp_rows[h], yTh[:])

            ph12.close()

            # ---------------- phase 3: out = y @ W_proj + b ----------------
            # all 4 n-chunks inner: each yt lhsT is reused by 4 adjacent mms
            with tc.tile_pool(name="psC", bufs=1, space="PSUM") as psC, \
                 tc.tile_pool(name="ph3", bufs=2) as ph3:
                wp = []
                for hh in range(NH if do_proj else 0):
                    wpc = ph3.tile([P, C], mm_dt, tag=f"wp{hh}", bufs=1,
                                   name=f"wp{hh}")
                    nc.sync.dma_start(wpc[:], r(wp_3d[:, hh, :]))
                    wp.append(wpc)
                for i in range(TT if do_proj else 0):
                    yt = ph3.tile([P, C], mm_dt, tag="yt")
                    nc.sync.dma_start(
                        yt[:].rearrange("p (hh f) -> p hh f", f=P),
                        r(ysp_3d[:, :, i * P:(i + 1) * P]))
                    po = [psC.tile([P, 512], F32, tag="big", bufs=6,
                                   name=f"po{i}_{nn}")
                          for nn in range(4)]
                    for hh in range(NH):
                        for nn in range(4):
                            nc.tensor.matmul(
                                po[nn][:], yt[:, hh * P:(hh + 1) * P],
                                wp[hh][:, nn * 512:(nn + 1) * 512],
                                start=(hh == 0), stop=False)
                    for nn in range(4):
                        nc.tensor.matmul(
                            po[nn][:], ones_row[:],
                            bp_sb[:, nn * 512:(nn + 1) * 512],
                            start=False, stop=True)
                        osb = ph3.tile([P, 512], F32, tag="osb")
                        nc.scalar.activation(
                            osb[:], po[nn][:],
                            mybir.ActivationFunctionType.Copy)
                        nc.sync.dma_start(
                            out_rows[i][:, nn * 512:(nn + 1) * 512], osb[:])
            rctx.close()

    return nc


_BUILT = {}


def _get_nc(mode: str):
    if mode not in _BUILT:
        _BUILT[mode] = build_kernel(mode)
    return _BUILT[mode]


def kernel(x, W_attn, b_attn, W_proj, b_proj, mode: str = "f32r", **run_kwargs):
    from concourse.bass_utils import run_bass_kernel_spmd

    x = np.asarray(x, dtype=np.float32)
    W_attn = np.ascontiguousarray(np.asarray(W_attn, dtype=np.float32))
    b_attn = np.ascontiguousarray(np.asarray(b_attn, dtype=np.float32))
    W_proj = np.ascontiguousarray(np.asarray(W_proj, dtype=np.float32))
    b_proj = np.ascontiguousarray(np.asarray(b_proj, dtype=np.float32))

    nc = _get_nc(mode)
    in_maps = [
        {
            "x": np.ascontiguousarray(x[b]),
            "W_attn": W_attn,
            "b_attn": b_attn,
            "W_proj": W_proj,
            "b_proj": b_proj,
        }
        for b in range(N_CORES)
    ]
    res = run_bass_kernel_spmd(nc, in_maps, list(range(N_CORES)), **run_kwargs)
    out = np.stack([res.results[b]["out"] for b in range(N_CORES)], axis=0)
    kernel.last_results = res
    return out

